# revision 37
# baseline (speedup 1.0000x reference)
"""Trainium2 Bass kernel for nn_CrossAttentionBlock (B=8, C=256, H=W=48).

Sharding: data-parallel over batch B — one batch per NeuronCore (8 cores).

Per-core math (x: [C=256, N=2304] f32):
  LayerNorm over C folded into projection weights on host:
      W_eff = W * w_n[None,:],  b_eff = b + W @ b_n
  attention SCALE folded into Wq_eff / bq_eff.
  v bias folded into the output bias (softmax rows sum to one, so
  attn@(v+bv) = attn@v + bv, hence bp_eff = bp + Wp@bv_eff).

  Activations are host-cast to bf16 and DMA'd as [32, N] partition strips
  (the DMA queues are descriptor-rate-bound; full-row descriptors maximize
  bytes per descriptor).  Stats: mean via a 1/C-ones bf16 matmul on x;
  mean-square via a Pool-computed x*x fed to a second ones-matmul.
  xn = (x-u)*rstd on the DVE, emitted as bf16.

  Attention is computed transposed:  St[m,n] = sum_o k[o,m] q[o,n]
  so softmax normalization runs over the *partition* axis m:
    - no row-max subtraction (logits bounded ~21, exp safe in f32)
    - P = exp(St) (ScalarE, PSUM->SBUF bf16 eviction)
    - rowsum[n] = sum_m P[m,n] via an M=1 ones-matmul accumulated across m
    - 1/rowsum applied AFTER the output projection (scaling commutes
      with Wp); the reciprocal row is partition-broadcast by the Pool
      engine and fused into the PSUM->SBUF eviction of the attention
      output, and the Wp projection + residual + DMA-out run per chunk
      inside the attention loop so there is no serial tail.
  v is produced directly transposed (vT[m,o] = sum_c xn2[c,m] WvT[c,o])
  so P.V contracts over m on partitions with zero PE transposes.
"""

import os
import sys
import types
import ctypes
import contextlib

sys.path.insert(0, "/opt/trn_rl_repo")

import numpy as np
import ml_dtypes

# ---------------------------------------------------------------------------
# NTFF profile hook stub (antenv.axon_hooks is absent in this container; the
# ctypes shim mirrors trn_agent_boot). Only used when tracing is requested.
# ---------------------------------------------------------------------------


def _ntff_profile_via_ctypes(so_path):
    try:
        lib = ctypes.CDLL(so_path)
    except OSError:
        return None
    if not hasattr(lib, "axon_start_nrt_profile"):
        return None
    lib.axon_start_nrt_profile.argtypes = [
        ctypes.POINTER(ctypes.c_int64),
        ctypes.c_size_t,
    ]
    lib.axon_start_nrt_profile.restype = ctypes.c_int64
    lib.axon_stop_nrt_profile.argtypes = [ctypes.c_char_p]
    lib.axon_stop_nrt_profile.restype = ctypes.c_int64

    @contextlib.contextmanager
    def _hook(output_dir, device_ids):
        import jax

        jax.devices()
        if device_ids:
            ids = (ctypes.c_int64 * len(device_ids))(*device_ids)
            rc = lib.axon_start_nrt_profile(ids, len(device_ids))
        else:
            rc = lib.axon_start_nrt_profile(None, 0)
        if rc != 0:
            raise RuntimeError(f"axon_start_nrt_profile rc={rc}")
        try:
            yield
        finally:
            n = lib.axon_stop_nrt_profile(str(output_dir).encode())
            print(f"profile: {n} file(s) written to {output_dir}", file=sys.stderr)

    return _hook


if "antenv.axon_hooks" not in sys.modules:
    _hook = _ntff_profile_via_ctypes("/opt/axon/libaxon_pjrt.so")
    _mod = types.ModuleType("antenv.axon_hooks")
    _mod.get_axon_ntff_profile_hook = lambda: _hook
    sys.modules["antenv.axon_hooks"] = _mod

# ---------------------------------------------------------------------------

B, C, H, W = 8, 256, 48, 48
N = H * W  # 2304
SCALE = (C // 8) ** (-0.5)
EPS = 1e-6
CT = C // 128  # 2 channel tiles
MT = N // 128  # 18 m (key-token) tiles
CHUNKS = [(0, 512), (512, 512), (1024, 512), (1536, 512), (2048, 256)]
NJ = len(CHUNKS)

BF16 = ml_dtypes.bfloat16

_cache = {}
last_results = None  # BassKernelResults of the most recent run (for test.py)


def _build_program():
    import concourse.bacc as bacc
    import concourse.tile as tile
    import concourse.mybir as mybir
    from contextlib import ExitStack

    f32 = mybir.dt.float32
    bf16 = mybir.dt.bfloat16
    ADD = mybir.AluOpType.add
    SUB = mybir.AluOpType.subtract

    nc = bacc.Bacc("TRN2", target_bir_lowering=False, debug=False)

    x1_d = nc.dram_tensor("x1", [C, N], bf16, kind="ExternalInput").ap()
    x2_d = nc.dram_tensor("x2", [C, N], bf16, kind="ExternalInput").ap()
    wqt_d = nc.dram_tensor("wqt", [C, C], bf16, kind="ExternalInput").ap()
    wkt_d = nc.dram_tensor("wkt", [C, C], bf16, kind="ExternalInput").ap()
    wvt_d = nc.dram_tensor("wvt", [C, C], bf16, kind="ExternalInput").ap()
    wpt_d = nc.dram_tensor("wpt", [C, C], bf16, kind="ExternalInput").ap()
    # cvec columns: 0/1 = bq per o-tile, 2/3 = bk per o-tile, 4/5 = bp_eff per
    # c-tile.  (The 1/C and ones constant blocks are memset on-device.)
    cvec_d = nc.dram_tensor("cvec", [128, 6], f32, kind="ExternalInput").ap()
    out_d = nc.dram_tensor("out", [C, N], f32, kind="ExternalOutput").ap()

    # m-tiles covered by each chunk: chunk j covers m in [off/128, (off+w)/128)
    def chunk_mtiles(ji):
        off, w = CHUNKS[ji]
        return range(off // 128, (off + w) // 128)

    with tile.TileContext(nc) as tc, ExitStack() as ctx:
        persist = ctx.enter_context(tc.tile_pool(name="persist", bufs=1))

        # DMA notes: (1) each descriptor covers one partition-row segment and
        # the queues are descriptor-rate-bound (~50ns each), so loads are
        # split into partition strips (full rows = max bytes per descriptor);
        # (2) each dma_start costs ~600ns of *issue* time on its engine's
        # sequencer, so issues are spread across the four idle sequencers.
        def strip_load(eng, dst, src, prows):
            P = dst.shape[0]
            for p in range(0, P, prows):
                pe = min(P, p + prows)
                eng.dma_start(dst[p:pe, :], src[p:pe, :])

        # ---- constants built on-device (no DMA) ------------------------
        invC = persist.tile([128, 128], bf16, tag="invC", name="invC")
        nc.vector.memset(invC[:], 1.0 / C)
        onesb = persist.tile([128, 128], bf16, tag="onesb", name="onesb")
        nc.gpsimd.memset(onesb[:], 1.0)

        # ---- x2 first: it gates k/v -> attention -----------------------
        x2sc = ctx.enter_context(tc.tile_pool(name="x2scope", bufs=1))
        x2_t = [
            x2sc.tile([128, N], bf16, tag=f"x2_{ct}", name=f"x2_{ct}")
            for ct in range(CT)
        ]
        strip_load(nc.sync, x2_t[0], x2_d[0:128, :], 64)
        strip_load(nc.scalar, x2_t[1], x2_d[128:256, :], 64)

        w_tiles = {}
        for (nm, d), eng in ((("k", wkt_d), nc.sync), (("v", wvt_d), nc.scalar)):
            for ct in range(CT):
                t = persist.tile([128, C], bf16, tag=f"w{nm}{ct}", name=f"w{nm}{ct}")
                strip_load(eng, t, d[ct * 128 : (ct + 1) * 128, :], 64)
                w_tiles[(nm, ct)] = t
        cvec = persist.tile([128, 6], f32, tag="cvec", name="cvec")
        nc.sync.dma_start(cvec[:], cvec_d[:, :])

        x1_t = [
            persist.tile([128, N], bf16, tag=f"x1_{ct}", name=f"x1_{ct}")
            for ct in range(CT)
        ]
        strip_load(nc.sync, x1_t[0], x1_d[0:128, :], 64)
        strip_load(nc.scalar, x1_t[1], x1_d[128:256, :], 64)

        for nm, d in (("q", wqt_d), ("p", wpt_d)):
            for ct in range(CT):
                t = persist.tile([128, C], bf16, tag=f"w{nm}{ct}", name=f"w{nm}{ct}")
                nc.gpsimd.dma_start(t[:], d[ct * 128 : (ct + 1) * 128, :])
                w_tiles[(nm, ct)] = t

        # persistent intermediates
        k_t = [
            persist.tile([128, N], bf16, tag=f"k{ot}", name=f"k{ot}")
            for ot in range(CT)
        ]
        vT_t = [
            persist.tile([128, C], bf16, tag=f"vT{m}", name=f"vT{m}")
            for m in range(MT)
        ]
        xn1_t = [
            persist.tile([128, N], bf16, tag=f"xn1_{ct}", name=f"xn1_{ct}")
            for ct in range(CT)
        ]

        # ------------------------------------------------------------------
        # Pre-phase: per-chunk pipeline  stats -> xn -> k/vT   (x2 stream)
        # plus the x1 stats/xn stream (feeds q projections later).
        # ------------------------------------------------------------------
        with (
            tc.tile_pool(name="scr", bufs=3) as scr,
            tc.tile_pool(name="xnp", bufs=6) as xnp,
            tc.tile_pool(name="ps_st", bufs=2, space="PSUM") as ps_st,
            tc.tile_pool(name="ps_kv", bufs=2, space="PSUM") as ps_kv,
        ):

            def emit_stats_xn(tsel, ji, xsrc, xn_out):
                """stats + xn for (tensor tsel, chunk ji).

                xsrc: list of [128, N] bf16 tiles (per ct)
                xn_out: dict key (ct) -> (tile, col_off) destination slices
                """
                off, w = CHUNKS[ji]
                ub = ps_st.tile([128, 512], f32, tag="ub", name="ub")
                for ct in range(CT):
                    nc.tensor.matmul(
                        ub[:, :w],
                        invC[:],
                        xsrc[ct][:, off : off + w],
                        start=(ct == 0),
                        stop=(ct == CT - 1),
                    )
                ms = ps_st.tile([128, 512], f32, tag="ms", name="ms")
                for ct in range(CT):
                    xsq = scr.tile([128, 512], bf16, tag="xsq", name="xsq")
                    nc.gpsimd.tensor_mul(
                        xsq[:, :w],
                        xsrc[ct][:, off : off + w],
                        xsrc[ct][:, off : off + w],
                    )
                    nc.tensor.matmul(
                        ms[:, :w],
                        invC[:],
                        xsq[:, :w],
                        start=(ct == 0),
                        stop=(ct == CT - 1),
                    )
                usq = scr.tile([128, 512], f32, tag="usq", name="usq")
                nc.scalar.square(usq[:, :w], ub[:, :w])
                var = scr.tile([128, 512], f32, tag="var", name="var")
                nc.vector.scalar_tensor_tensor(
                    var[:, :w], ms[:, :w], EPS, usq[:, :w], ADD, SUB
                )
                std = scr.tile([128, 512], f32, tag="std", name="std")
                nc.scalar.activation(
                    std[:, :w], var[:, :w], mybir.ActivationFunctionType.Sqrt
                )
                rstd = scr.tile([128, 512], f32, tag=f"rstd{tsel}", name=f"rstd{tsel}")
                nc.vector.reciprocal_approx_fast(rstd[:, :w], std[:, :w])
                for ct in range(CT):
                    d = scr.tile([128, 512], f32, tag="xnd", name="xnd")
                    nc.vector.tensor_sub(
                        d[:, :w], xsrc[ct][:, off : off + w], ub[:, :w]
                    )
                    dst, dcol = xn_out[ct]
                    # x1's multiplies go to the otherwise-idle Pool engine
                    eng = nc.vector if tsel == 1 else nc.gpsimd
                    eng.tensor_mul(dst[:, dcol : dcol + w], d[:, :w], rstd[:, :w])

            xn2 = {}

            def emit_kv(ji):
                off, w = CHUNKS[ji]
                # k projection for this chunk of tokens
                for ot in range(CT):
                    ps = ps_kv.tile([128, 512], f32, tag="kv", name="kv")
                    for ct in range(CT):
                        nc.tensor.matmul(
                            ps[:, :w],
                            w_tiles[("k", ct)][:, ot * 128 : (ot + 1) * 128],
                            xn2[(ji, ct)][:, :w],
                            start=(ct == 0),
                            stop=(ct == CT - 1),
                        )
                    nc.vector.tensor_scalar_add(
                        k_t[ot][:, off : off + w], ps[:, :w], cvec[:, 2 + ot : 3 + ot]
                    )
                # vT for the m-tiles inside this chunk
                for m in chunk_mtiles(ji):
                    coff = m * 128 - off
                    ps = ps_kv.tile([128, C], f32, tag="kv", name="kv")
                    for ct in range(CT):
                        nc.tensor.matmul(
                            ps[:],
                            xn2[(ji, ct)][:, coff : coff + 128],
                            w_tiles[("v", ct)][:, :],
                            start=(ct == 0),
                            stop=(ct == CT - 1),
                        )
                    nc.scalar.copy(vT_t[m][:], ps[:])

            # x2 stream first (it gates the attention m-loop chunk by chunk);
            # x1 chunk 0 next (it gates qproj(0) and thus attention start);
            # x1 chunks 1-4 are demoted BELOW the attention loop's priority
            # so they only fill engine bubbles during attention.
            for ji in range(NJ):
                for ct in range(CT):
                    t = xnp.tile([128, 512], bf16, tag="xn2", name=f"xn2_{ji}_{ct}")
                    xn2[(ji, ct)] = t
                emit_stats_xn(1, ji, x2_t, {ct: (xn2[(ji, ct)], 0) for ct in range(CT)})
                emit_kv(ji)
                if ji == 0:
                    emit_stats_xn(
                        0, 0, x1_t, {ct: (xn1_t[ct], 0) for ct in range(CT)}
                    )
            with tc.high_priority(offset=-(10**6)):
                for ji in range(1, NJ):
                    emit_stats_xn(
                        0, ji, x1_t,
                        {ct: (xn1_t[ct], CHUNKS[ji][0]) for ct in range(CT)},
                    )

        # ------------------------------------------------------------------
        # Attention: per q-chunk; q projected one chunk ahead; epilogue
        # (normalize, Wp projection, residual, DMA out) inside the loop.
        # ------------------------------------------------------------------
        with (
            tc.tile_pool(name="qch", bufs=4) as qch,
            tc.tile_pool(name="pt", bufs=24) as pt_pool,
            tc.tile_pool(name="oup", bufs=4) as oup,
            tc.tile_pool(name="invp", bufs=2) as invp,
            tc.tile_pool(name="outp", bufs=4) as outp,
            tc.tile_pool(name="ps_pj", bufs=2, space="PSUM") as ps_pj,
            tc.tile_pool(name="ps_qk", bufs=2, space="PSUM") as ps_qk,
            tc.tile_pool(name="ps_o", bufs=3, space="PSUM") as ps_o,
            tc.tile_pool(name="ps_rs", bufs=1, space="PSUM") as ps_rs,
        ):
            q_ch = {}

            def emit_qproj(ji):
                off, w = CHUNKS[ji]
                for ot in range(CT):
                    ps = ps_pj.tile([128, 512], f32, tag="pj", name="pj")
                    for ct in range(CT):
                        nc.tensor.matmul(
                            ps[:, :w],
                            w_tiles[("q", ct)][:, ot * 128 : (ot + 1) * 128],
                            xn1_t[ct][:, off : off + w],
                            start=(ct == 0),
                            stop=(ct == CT - 1),
                        )
                    qt = qch.tile([128, 512], bf16, tag="q", name=f"q{ji}_{ot}")
                    nc.vector.tensor_scalar_add(
                        qt[:, :w], ps[:, :w], cvec[:, 0 + ot : 1 + ot]
                    )
                    q_ch[(ji, ot)] = qt

            emit_qproj(0)
            for ji, (off, w) in enumerate(CHUNKS):
                if ji + 1 < NJ:
                    emit_qproj(ji + 1)
                st = {}

                def emit_qk(m):
                    ps = ps_qk.tile([128, 512], f32, tag="st", name="st")
                    for ot in range(CT):
                        nc.tensor.matmul(
                            ps[:, :w],
                            k_t[ot][:, m * 128 : (m + 1) * 128],
                            q_ch[(ji, ot)][:, :w],
                            start=(ot == 0),
                            stop=(ot == CT - 1),
                        )
                    st[m] = ps

                o_ps = [
                    ps_o.tile([128, 512], f32, tag="o", name="o") for _ in range(CT)
                ]

                # Rowsum: last chunk interleaves per-m ones-matmuls (keeps
                # the tail short); other chunks reduce the pt tiles with a
                # 2-level bf16 pair/quad tree on the otherwise-idle Pool
                # engine, leaving only 5 ones-matmuls per chunk on the PE.
                inline_rs = ji == NJ - 1
                rs_ps = ps_rs.tile([128, 512], f32, tag="rsp", name="rsp")

                pts = []
                pairs = []
                quads = []

                def emit_rs_tree(m):
                    if m % 2 == 1:
                        pr = pt_pool.tile([128, 512], bf16, tag="pr", name=f"pr{m}")
                        nc.vector.tensor_add(
                            pr[:, :w], pts[m - 1][:, :w], pts[m][:, :w]
                        )
                        pairs.append(pr)

                emit_qk(0)
                for m in range(MT):
                    if m + 1 < MT:
                        emit_qk(m + 1)
                    pt = pt_pool.tile([128, 512], bf16, tag="pt", name=f"pt{m}")
                    nc.scalar.activation(
                        pt[:, :w], st[m][:, :w], mybir.ActivationFunctionType.Exp
                    )
                    del st[m]
                    pts.append(pt)
                    for c in range(CT):
                        nc.tensor.matmul(
                            o_ps[c][:, :w],
                            vT_t[m][:, c * 128 : (c + 1) * 128],
                            pt[:, :w],
                            start=(m == 0),
                            stop=(m == MT - 1),
                        )
                    if inline_rs:
                        nc.tensor.matmul(
                            rs_ps[:, :w],
                            onesb[:, 0:128],
                            pt[:, :w],
                            start=(m == 0),
                            stop=(m == MT - 1),
                        )
                    else:
                        emit_rs_tree(m)

                # ---- chunk epilogue -----------------------------------
                if not inline_rs:
                    toprs = pairs
                    for i, t in enumerate(toprs):
                        nc.tensor.matmul(
                            rs_ps[:, :w],
                            onesb[:, 0:128],
                            t[:, :w],
                            start=(i == 0),
                            stop=(i == len(toprs) - 1),
                        )
                inv_b = invp.tile([128, 512], f32, tag="invb", name="invb")
                nc.vector.reciprocal_approx_fast(inv_b[:, :w], rs_ps[:, :w])

                ou = []
                for c in range(CT):
                    t = oup.tile([128, 512], bf16, tag="ou", name=f"ou{c}")
                    nc.vector.tensor_mul(t[:, :w], o_ps[c][:, :w], inv_b[:, :w])
                    ou.append(t)

                for ct in range(CT):
                    ps = ps_pj.tile([128, 512], f32, tag="pj", name="pj")
                    for ci in range(CT):
                        nc.tensor.matmul(
                            ps[:, :w],
                            w_tiles[("p", ci)][:, ct * 128 : (ct + 1) * 128],
                            ou[ci][:, :w],
                            start=(ci == 0),
                            stop=(ci == CT - 1),
                        )
                    ot_t = outp.tile([128, 512], f32, tag="outt", name=f"out{ct}")
                    nc.vector.scalar_tensor_tensor(
                        ot_t[:, :w],
                        ps[:, :w],
                        cvec[:, 4 + ct : 5 + ct],
                        x1_t[ct][:, off : off + w],
                        ADD,
                        ADD,
                    )
                    if ji + 1 < NJ:
                        nc.sync.dma_start(
                            out_d[ct * 128 : (ct + 1) * 128, off : off + w],
                            ot_t[:, :w],
                        )
                    else:
                        # last chunk is latency-critical: strip across queues
                        # and split the issue cost across two sequencers
                        eng = nc.sync if ct == 0 else nc.scalar
                        for p in range(0, 128, 64):
                            eng.dma_start(
                                out_d[ct * 128 + p : ct * 128 + p + 64, off : off + w],
                                ot_t[p : p + 64, :w],
                            )

    nc.compile()
    return nc


def _host_prep(inputs):
    f = lambda k: np.asarray(inputs[k], dtype=np.float32)
    Wq, Wk, Wv, Wp = f("Wq"), f("Wk"), f("Wv"), f("Wp")
    bq, bk, bv, bp = f("bq"), f("bk"), f("bv"), f("bp")
    w_nq, b_nq, w_nkv, b_nkv = f("w_nq"), f("b_nq"), f("w_nkv"), f("b_nkv")

    Wq_eff = Wq * w_nq[None, :] * SCALE
    bq_eff = SCALE * (bq + Wq @ b_nq)
    Wk_eff = Wk * w_nkv[None, :]
    bk_eff = bk + Wk @ b_nkv
    Wv_eff = Wv * w_nkv[None, :]
    bv_eff = bv + Wv @ b_nkv
    bp_eff = bp + Wp @ bv_eff  # v bias folded through softmax + Wp

    wqt = np.ascontiguousarray(Wq_eff.T).astype(BF16)
    wkt = np.ascontiguousarray(Wk_eff.T).astype(BF16)
    wvt = np.ascontiguousarray(Wv_eff.T).astype(BF16)
    wpt = np.ascontiguousarray(Wp.T).astype(BF16)

    cvec = np.zeros((128, 6), np.float32)
    cvec[:, 0] = bq_eff[0:128]
    cvec[:, 1] = bq_eff[128:256]
    cvec[:, 2] = bk_eff[0:128]
    cvec[:, 3] = bk_eff[128:256]
    cvec[:, 4] = bp_eff[0:128]
    cvec[:, 5] = bp_eff[128:256]

    return dict(wqt=wqt, wkt=wkt, wvt=wvt, wpt=wpt, cvec=cvec)


def _maybe_patch_ldw_opt():
    if os.environ.get("BASS_LDW_OPT", "0") != "1":
        return
    import concourse.bass_utils as bu
    if getattr(bu, "_ldw_patch", False):
        return
    orig = bu.run_command
    def patched(argv, **kw):
        if isinstance(argv, list):
            argv = [a.replace("--enable-ldw-opt=false", "--enable-ldw-opt=true") for a in argv]
        return orig(argv, **kw)
    bu.run_command = patched
    bu._ldw_patch = True


def kernel(**inputs):
    global last_results
    _maybe_patch_ldw_opt()
    from concourse.bass_utils import run_bass_kernel_spmd

    if "nc" not in _cache:
        _cache["nc"] = _build_program()
    nc = _cache["nc"]

    shared = _host_prep(inputs)
    x1 = np.asarray(inputs["x1"], dtype=np.float32).reshape(B, C, N).astype(BF16)
    x2 = np.asarray(inputs["x2"], dtype=np.float32).reshape(B, C, N).astype(BF16)

    in_maps = []
    for b in range(B):
        m = dict(shared)
        m["x1"] = np.ascontiguousarray(x1[b])
        m["x2"] = np.ascontiguousarray(x2[b])
        in_maps.append(m)

    trace = os.environ.get("BASS_KERNEL_TRACE", "0") == "1"
    res = run_bass_kernel_spmd(
        nc, in_maps, core_ids=list(range(B)), trace=trace
    )
    last_results = res
    out = np.stack([res.results[b]["out"].reshape(C, H, W) for b in range(B)])
    return out.astype(np.float32)


# revision 38
# speedup vs baseline: 1.1267x; 1.1267x over previous
"""Trainium2 Bass kernel for nn_CrossAttentionBlock (B=8, C=256, H=W=48).

Sharding: data-parallel over batch B — one batch per NeuronCore (8 cores).

Per-core math (x: [C=256, N=2304] f32):
  LayerNorm over C folded into projection weights on host:
      W_eff = W * w_n[None,:],  b_eff = b + W @ b_n
  attention SCALE folded into Wq_eff / bq_eff.
  v bias folded into the output bias (softmax rows sum to one, so
  attn@(v+bv) = attn@v + bv, hence bp_eff = bp + Wp@bv_eff).

  Activations are host-cast to bf16 and DMA'd as [32, N] partition strips
  (the DMA queues are descriptor-rate-bound; full-row descriptors maximize
  bytes per descriptor).  Stats: mean via a 1/C-ones bf16 matmul on x;
  mean-square via a Pool-computed x*x fed to a second ones-matmul.
  xn = (x-u)*rstd on the DVE, emitted as bf16.

  Attention is computed transposed:  St[m,n] = sum_o k[o,m] q[o,n]
  so softmax normalization runs over the *partition* axis m:
    - no row-max subtraction (logits bounded ~21, exp safe in f32)
    - P = exp(St) (ScalarE, PSUM->SBUF bf16 eviction)
    - rowsum[n] = sum_m P[m,n] via an M=1 ones-matmul accumulated across m
    - 1/rowsum applied AFTER the output projection (scaling commutes
      with Wp); the reciprocal row is partition-broadcast by the Pool
      engine and fused into the PSUM->SBUF eviction of the attention
      output, and the Wp projection + residual + DMA-out run per chunk
      inside the attention loop so there is no serial tail.
  v is produced directly transposed (vT[m,o] = sum_c xn2[c,m] WvT[c,o])
  so P.V contracts over m on partitions with zero PE transposes.
"""

import os
import sys
import types
import ctypes
import contextlib

sys.path.insert(0, "/opt/trn_rl_repo")

import numpy as np
import ml_dtypes

# ---------------------------------------------------------------------------
# NTFF profile hook stub (antenv.axon_hooks is absent in this container; the
# ctypes shim mirrors trn_agent_boot). Only used when tracing is requested.
# ---------------------------------------------------------------------------


def _ntff_profile_via_ctypes(so_path):
    try:
        lib = ctypes.CDLL(so_path)
    except OSError:
        return None
    if not hasattr(lib, "axon_start_nrt_profile"):
        return None
    lib.axon_start_nrt_profile.argtypes = [
        ctypes.POINTER(ctypes.c_int64),
        ctypes.c_size_t,
    ]
    lib.axon_start_nrt_profile.restype = ctypes.c_int64
    lib.axon_stop_nrt_profile.argtypes = [ctypes.c_char_p]
    lib.axon_stop_nrt_profile.restype = ctypes.c_int64

    @contextlib.contextmanager
    def _hook(output_dir, device_ids):
        import jax

        jax.devices()
        if device_ids:
            ids = (ctypes.c_int64 * len(device_ids))(*device_ids)
            rc = lib.axon_start_nrt_profile(ids, len(device_ids))
        else:
            rc = lib.axon_start_nrt_profile(None, 0)
        if rc != 0:
            raise RuntimeError(f"axon_start_nrt_profile rc={rc}")
        try:
            yield
        finally:
            n = lib.axon_stop_nrt_profile(str(output_dir).encode())
            print(f"profile: {n} file(s) written to {output_dir}", file=sys.stderr)

    return _hook


if "antenv.axon_hooks" not in sys.modules:
    _hook = _ntff_profile_via_ctypes("/opt/axon/libaxon_pjrt.so")
    _mod = types.ModuleType("antenv.axon_hooks")
    _mod.get_axon_ntff_profile_hook = lambda: _hook
    sys.modules["antenv.axon_hooks"] = _mod

# ---------------------------------------------------------------------------

B, C, H, W = 8, 256, 48, 48
N = H * W  # 2304
SCALE = (C // 8) ** (-0.5)
EPS = 1e-6
CT = C // 128  # 2 channel tiles
MT = N // 128  # 18 m (key-token) tiles
CHUNKS = [(0, 512), (512, 512), (1024, 512), (1536, 512), (2048, 256)]
NJ = len(CHUNKS)

BF16 = ml_dtypes.bfloat16

_cache = {}
last_results = None  # BassKernelResults of the most recent run (for test.py)


def _build_program():
    import concourse.bacc as bacc
    import concourse.tile as tile
    import concourse.mybir as mybir
    from contextlib import ExitStack

    f32 = mybir.dt.float32
    bf16 = mybir.dt.bfloat16
    ADD = mybir.AluOpType.add
    SUB = mybir.AluOpType.subtract

    nc = bacc.Bacc("TRN2", target_bir_lowering=False, debug=False)

    x1_d = nc.dram_tensor("x1", [C, N], bf16, kind="ExternalInput").ap()
    x2_d = nc.dram_tensor("x2", [C, N], bf16, kind="ExternalInput").ap()
    wqt_d = nc.dram_tensor("wqt", [C, C], bf16, kind="ExternalInput").ap()
    wkt_d = nc.dram_tensor("wkt", [C, C], bf16, kind="ExternalInput").ap()
    wvt_d = nc.dram_tensor("wvt", [C, C], bf16, kind="ExternalInput").ap()
    wpt_d = nc.dram_tensor("wpt", [C, C], bf16, kind="ExternalInput").ap()
    # cvec columns: 0/1 = bq per o-tile, 2/3 = bk per o-tile, 4/5 = bp_eff per
    # c-tile.  (The 1/C and ones constant blocks are memset on-device.)
    cvec_d = nc.dram_tensor("cvec", [128, 6], f32, kind="ExternalInput").ap()
    out_d = nc.dram_tensor("out", [C, N], f32, kind="ExternalOutput").ap()

    # m-tiles covered by each chunk: chunk j covers m in [off/128, (off+w)/128)
    def chunk_mtiles(ji):
        off, w = CHUNKS[ji]
        return range(off // 128, (off + w) // 128)

    with tile.TileContext(nc) as tc, ExitStack() as ctx:
        persist = ctx.enter_context(tc.tile_pool(name="persist", bufs=1))

        # DMA notes: (1) each descriptor covers one partition-row segment and
        # the queues are descriptor-rate-bound (~50ns each), so loads are
        # split into partition strips (full rows = max bytes per descriptor);
        # (2) each dma_start costs ~600ns of *issue* time on its engine's
        # sequencer, so issues are spread across the four idle sequencers.
        def strip_load(eng, dst, src, prows):
            P = dst.shape[0]
            for p in range(0, P, prows):
                pe = min(P, p + prows)
                eng.dma_start(dst[p:pe, :], src[p:pe, :])

        # ---- constants built on-device (no DMA) ------------------------
        invC = persist.tile([128, 128], bf16, tag="invC", name="invC")
        nc.vector.memset(invC[:], 1.0 / C)
        onesb = persist.tile([128, 128], bf16, tag="onesb", name="onesb")
        nc.gpsimd.memset(onesb[:], 1.0)

        # ---- x2 first: it gates k/v -> attention -----------------------
        x2sc = ctx.enter_context(tc.tile_pool(name="x2scope", bufs=1))
        x2_t = [
            x2sc.tile([128, N], bf16, tag=f"x2_{ct}", name=f"x2_{ct}")
            for ct in range(CT)
        ]
        strip_load(nc.sync, x2_t[0], x2_d[0:128, :], 64)
        strip_load(nc.scalar, x2_t[1], x2_d[128:256, :], 64)

        w_tiles = {}
        for (nm, d), eng in ((("k", wkt_d), nc.sync), (("v", wvt_d), nc.scalar)):
            for ct in range(CT):
                t = persist.tile([128, C], bf16, tag=f"w{nm}{ct}", name=f"w{nm}{ct}")
                strip_load(eng, t, d[ct * 128 : (ct + 1) * 128, :], 64)
                w_tiles[(nm, ct)] = t
        cvec = persist.tile([128, 6], f32, tag="cvec", name="cvec")
        nc.sync.dma_start(cvec[:], cvec_d[:, :])

        x1_t = [
            persist.tile([128, N], bf16, tag=f"x1_{ct}", name=f"x1_{ct}")
            for ct in range(CT)
        ]
        strip_load(nc.sync, x1_t[0], x1_d[0:128, :], 64)
        strip_load(nc.scalar, x1_t[1], x1_d[128:256, :], 64)

        for nm, d in (("q", wqt_d), ("p", wpt_d)):
            for ct in range(CT):
                t = persist.tile([128, C], bf16, tag=f"w{nm}{ct}", name=f"w{nm}{ct}")
                nc.gpsimd.dma_start(t[:], d[ct * 128 : (ct + 1) * 128, :])
                w_tiles[(nm, ct)] = t

        # persistent intermediates
        k_t = [
            persist.tile([128, N], bf16, tag=f"k{ot}", name=f"k{ot}")
            for ot in range(CT)
        ]
        vT_t = [
            persist.tile([128, C], bf16, tag=f"vT{m}", name=f"vT{m}")
            for m in range(MT)
        ]
        xn1_t = [
            persist.tile([128, N], bf16, tag=f"xn1_{ct}", name=f"xn1_{ct}")
            for ct in range(CT)
        ]

        # ------------------------------------------------------------------
        # Pre-phase: per-chunk pipeline  stats -> xn -> k/vT   (x2 stream)
        # plus the x1 stats/xn stream (feeds q projections later).
        # ------------------------------------------------------------------
        with (
            tc.tile_pool(name="scr", bufs=3) as scr,
            tc.tile_pool(name="xnp", bufs=6) as xnp,
            tc.tile_pool(name="ps_st", bufs=2, space="PSUM") as ps_st,
            tc.tile_pool(name="ps_kv", bufs=2, space="PSUM") as ps_kv,
        ):

            def emit_stats_xn(tsel, ji, xsrc, xn_out):
                """stats + xn for (tensor tsel, chunk ji).

                xsrc: list of [128, N] bf16 tiles (per ct)
                xn_out: dict key (ct) -> (tile, col_off) destination slices
                """
                off, w = CHUNKS[ji]
                ub = ps_st.tile([128, 512], f32, tag="ub", name="ub")
                for ct in range(CT):
                    nc.tensor.matmul(
                        ub[:, :w],
                        invC[:],
                        xsrc[ct][:, off : off + w],
                        start=(ct == 0),
                        stop=(ct == CT - 1),
                    )
                ms = ps_st.tile([128, 512], f32, tag="ms", name="ms")
                for ct in range(CT):
                    xsq = scr.tile([128, 512], bf16, tag="xsq", name="xsq")
                    nc.gpsimd.tensor_mul(
                        xsq[:, :w],
                        xsrc[ct][:, off : off + w],
                        xsrc[ct][:, off : off + w],
                    )
                    nc.tensor.matmul(
                        ms[:, :w],
                        invC[:],
                        xsq[:, :w],
                        start=(ct == 0),
                        stop=(ct == CT - 1),
                    )
                usq = scr.tile([128, 512], f32, tag="usq", name="usq")
                nc.scalar.square(usq[:, :w], ub[:, :w])
                var = scr.tile([128, 512], f32, tag="var", name="var")
                nc.vector.scalar_tensor_tensor(
                    var[:, :w], ms[:, :w], EPS, usq[:, :w], ADD, SUB
                )
                std = scr.tile([128, 512], f32, tag="std", name="std")
                nc.scalar.activation(
                    std[:, :w], var[:, :w], mybir.ActivationFunctionType.Sqrt
                )
                rstd = scr.tile([128, 512], f32, tag=f"rstd{tsel}", name=f"rstd{tsel}")
                nc.vector.reciprocal_approx_fast(rstd[:, :w], std[:, :w])
                for ct in range(CT):
                    d = scr.tile([128, 512], f32, tag="xnd", name="xnd")
                    nc.vector.tensor_sub(
                        d[:, :w], xsrc[ct][:, off : off + w], ub[:, :w]
                    )
                    dst, dcol = xn_out[ct]
                    # x1's multiplies go to the otherwise-idle Pool engine
                    eng = nc.vector if tsel == 1 else nc.gpsimd
                    eng.tensor_mul(dst[:, dcol : dcol + w], d[:, :w], rstd[:, :w])

            xn2 = {}

            def emit_kv(ji):
                off, w = CHUNKS[ji]
                # k projection for this chunk of tokens
                for ot in range(CT):
                    ps = ps_kv.tile([128, 512], f32, tag="kv", name="kv")
                    for ct in range(CT):
                        nc.tensor.matmul(
                            ps[:, :w],
                            w_tiles[("k", ct)][:, ot * 128 : (ot + 1) * 128],
                            xn2[(ji, ct)][:, :w],
                            start=(ct == 0),
                            stop=(ct == CT - 1),
                        )
                    nc.vector.tensor_scalar_add(
                        k_t[ot][:, off : off + w], ps[:, :w], cvec[:, 2 + ot : 3 + ot]
                    )
                # vT for the m-tiles inside this chunk
                for m in chunk_mtiles(ji):
                    coff = m * 128 - off
                    ps = ps_kv.tile([128, C], f32, tag="kv", name="kv")
                    for ct in range(CT):
                        nc.tensor.matmul(
                            ps[:],
                            xn2[(ji, ct)][:, coff : coff + 128],
                            w_tiles[("v", ct)][:, :],
                            start=(ct == 0),
                            stop=(ct == CT - 1),
                        )
                    nc.scalar.copy(vT_t[m][:], ps[:])

            # x2 stream first (it gates the attention m-loop chunk by chunk);
            # x1 chunk 0 next (it gates qproj(0) and thus attention start);
            # x1 chunks 1-4 are demoted BELOW the attention loop's priority
            # so they only fill engine bubbles during attention.
            for ji in range(NJ):
                for ct in range(CT):
                    t = xnp.tile([128, 512], bf16, tag="xn2", name=f"xn2_{ji}_{ct}")
                    xn2[(ji, ct)] = t
                emit_stats_xn(1, ji, x2_t, {ct: (xn2[(ji, ct)], 0) for ct in range(CT)})
                emit_kv(ji)
                if ji == 0:
                    emit_stats_xn(
                        0, 0, x1_t, {ct: (xn1_t[ct], 0) for ct in range(CT)}
                    )
            with tc.high_priority(offset=-(10**6)):
                for ji in range(1, NJ):
                    emit_stats_xn(
                        0, ji, x1_t,
                        {ct: (xn1_t[ct], CHUNKS[ji][0]) for ct in range(CT)},
                    )

        # ------------------------------------------------------------------
        # Attention: per q-chunk; q projected one chunk ahead; epilogue
        # (normalize, Wp projection, residual, DMA out) inside the loop.
        # ------------------------------------------------------------------
        with (
            tc.tile_pool(name="qch", bufs=4) as qch,
            tc.tile_pool(name="pt", bufs=24) as pt_pool,
            tc.tile_pool(name="oup", bufs=4) as oup,
            tc.tile_pool(name="invp", bufs=2) as invp,
            tc.tile_pool(name="outp", bufs=4) as outp,
            tc.tile_pool(name="ps_pj", bufs=2, space="PSUM") as ps_pj,
            tc.tile_pool(name="ps_qk", bufs=2, space="PSUM") as ps_qk,
            tc.tile_pool(name="ps_o", bufs=3, space="PSUM") as ps_o,
            tc.tile_pool(name="ps_rs", bufs=1, space="PSUM") as ps_rs,
        ):
            q_ch = {}

            def emit_qproj(ji):
                off, w = CHUNKS[ji]
                for ot in range(CT):
                    ps = ps_pj.tile([128, 512], f32, tag="pj", name="pj")
                    for ct in range(CT):
                        nc.tensor.matmul(
                            ps[:, :w],
                            w_tiles[("q", ct)][:, ot * 128 : (ot + 1) * 128],
                            xn1_t[ct][:, off : off + w],
                            start=(ct == 0),
                            stop=(ct == CT - 1),
                        )
                    qt = qch.tile([128, 512], bf16, tag="q", name=f"q{ji}_{ot}")
                    nc.vector.tensor_scalar_add(
                        qt[:, :w], ps[:, :w], cvec[:, 0 + ot : 1 + ot]
                    )
                    q_ch[(ji, ot)] = qt

            emit_qproj(0)
            for ji, (off, w) in enumerate(CHUNKS):
                if ji + 1 < NJ:
                    emit_qproj(ji + 1)
                st = {}

                def emit_qk(m):
                    ps = ps_qk.tile([128, 512], f32, tag="st", name="st")
                    for ot in range(CT):
                        nc.tensor.matmul(
                            ps[:, :w],
                            k_t[ot][:, m * 128 : (m + 1) * 128],
                            q_ch[(ji, ot)][:, :w],
                            start=(ot == 0),
                            stop=(ot == CT - 1),
                        )
                    st[m] = ps

                o_ps = [
                    ps_o.tile([128, 512], f32, tag="o", name="o") for _ in range(CT)
                ]

                # Rowsum: last chunk interleaves per-m ones-matmuls (keeps
                # the tail short); other chunks reduce the pt tiles with a
                # 2-level bf16 pair/quad tree on the otherwise-idle Pool
                # engine, leaving only 5 ones-matmuls per chunk on the PE.
                inline_rs = ji == NJ - 1
                rs_ps = ps_rs.tile([128, 512], f32, tag="rsp", name="rsp")

                pts = []

                emit_qk(0)
                for m in range(MT):
                    if m + 1 < MT:
                        emit_qk(m + 1)
                    pt = pt_pool.tile([128, 512], bf16, tag="pt", name=f"pt{m}")
                    nc.scalar.activation(
                        pt[:, :w], st[m][:, :w], mybir.ActivationFunctionType.Exp
                    )
                    del st[m]
                    pts.append(pt)
                    for c in range(CT):
                        nc.tensor.matmul(
                            o_ps[c][:, :w],
                            vT_t[m][:, c * 128 : (c + 1) * 128],
                            pt[:, :w],
                            start=(m == 0),
                            stop=(m == MT - 1),
                        )
                    if inline_rs:
                        nc.tensor.matmul(
                            rs_ps[:, :w],
                            onesb[:, 0:128],
                            pt[:, :w],
                            start=(m == 0),
                            stop=(m == MT - 1),
                        )

                # ---- chunk epilogue -----------------------------------
                if not inline_rs:
                    for i, t in enumerate(pts):
                        nc.tensor.matmul(
                            rs_ps[:, :w],
                            onesb[:, 0:128],
                            t[:, :w],
                            start=(i == 0),
                            stop=(i == len(pts) - 1),
                        )
                inv_b = invp.tile([128, 512], f32, tag="invb", name="invb")
                nc.vector.reciprocal_approx_fast(inv_b[:, :w], rs_ps[:, :w])

                ou = []
                for c in range(CT):
                    t = oup.tile([128, 512], bf16, tag="ou", name=f"ou{c}")
                    nc.vector.tensor_mul(t[:, :w], o_ps[c][:, :w], inv_b[:, :w])
                    ou.append(t)

                for ct in range(CT):
                    ps = ps_pj.tile([128, 512], f32, tag="pj", name="pj")
                    for ci in range(CT):
                        nc.tensor.matmul(
                            ps[:, :w],
                            w_tiles[("p", ci)][:, ct * 128 : (ct + 1) * 128],
                            ou[ci][:, :w],
                            start=(ci == 0),
                            stop=(ci == CT - 1),
                        )
                    ot_t = outp.tile([128, 512], f32, tag="outt", name=f"out{ct}")
                    nc.vector.scalar_tensor_tensor(
                        ot_t[:, :w],
                        ps[:, :w],
                        cvec[:, 4 + ct : 5 + ct],
                        x1_t[ct][:, off : off + w],
                        ADD,
                        ADD,
                    )
                    if ji + 1 < NJ:
                        nc.sync.dma_start(
                            out_d[ct * 128 : (ct + 1) * 128, off : off + w],
                            ot_t[:, :w],
                        )
                    else:
                        # last chunk is latency-critical: strip across queues
                        # and split the issue cost across two sequencers
                        eng = nc.sync if ct == 0 else nc.scalar
                        for p in range(0, 128, 64):
                            eng.dma_start(
                                out_d[ct * 128 + p : ct * 128 + p + 64, off : off + w],
                                ot_t[p : p + 64, :w],
                            )

    nc.compile()
    return nc


def _host_prep(inputs):
    f = lambda k: np.asarray(inputs[k], dtype=np.float32)
    Wq, Wk, Wv, Wp = f("Wq"), f("Wk"), f("Wv"), f("Wp")
    bq, bk, bv, bp = f("bq"), f("bk"), f("bv"), f("bp")
    w_nq, b_nq, w_nkv, b_nkv = f("w_nq"), f("b_nq"), f("w_nkv"), f("b_nkv")

    Wq_eff = Wq * w_nq[None, :] * SCALE
    bq_eff = SCALE * (bq + Wq @ b_nq)
    Wk_eff = Wk * w_nkv[None, :]
    bk_eff = bk + Wk @ b_nkv
    Wv_eff = Wv * w_nkv[None, :]
    bv_eff = bv + Wv @ b_nkv
    bp_eff = bp + Wp @ bv_eff  # v bias folded through softmax + Wp

    wqt = np.ascontiguousarray(Wq_eff.T).astype(BF16)
    wkt = np.ascontiguousarray(Wk_eff.T).astype(BF16)
    wvt = np.ascontiguousarray(Wv_eff.T).astype(BF16)
    wpt = np.ascontiguousarray(Wp.T).astype(BF16)

    cvec = np.zeros((128, 6), np.float32)
    cvec[:, 0] = bq_eff[0:128]
    cvec[:, 1] = bq_eff[128:256]
    cvec[:, 2] = bk_eff[0:128]
    cvec[:, 3] = bk_eff[128:256]
    cvec[:, 4] = bp_eff[0:128]
    cvec[:, 5] = bp_eff[128:256]

    return dict(wqt=wqt, wkt=wkt, wvt=wvt, wpt=wpt, cvec=cvec)


def _maybe_patch_ldw_opt():
    if os.environ.get("BASS_LDW_OPT", "0") != "1":
        return
    import concourse.bass_utils as bu
    if getattr(bu, "_ldw_patch", False):
        return
    orig = bu.run_command
    def patched(argv, **kw):
        if isinstance(argv, list):
            argv = [a.replace("--enable-ldw-opt=false", "--enable-ldw-opt=true") for a in argv]
        return orig(argv, **kw)
    bu.run_command = patched
    bu._ldw_patch = True


def kernel(**inputs):
    global last_results
    _maybe_patch_ldw_opt()
    from concourse.bass_utils import run_bass_kernel_spmd

    if "nc" not in _cache:
        _cache["nc"] = _build_program()
    nc = _cache["nc"]

    shared = _host_prep(inputs)
    x1 = np.asarray(inputs["x1"], dtype=np.float32).reshape(B, C, N).astype(BF16)
    x2 = np.asarray(inputs["x2"], dtype=np.float32).reshape(B, C, N).astype(BF16)

    in_maps = []
    for b in range(B):
        m = dict(shared)
        m["x1"] = np.ascontiguousarray(x1[b])
        m["x2"] = np.ascontiguousarray(x2[b])
        in_maps.append(m)

    trace = os.environ.get("BASS_KERNEL_TRACE", "0") == "1"
    res = run_bass_kernel_spmd(
        nc, in_maps, core_ids=list(range(B)), trace=trace
    )
    last_results = res
    out = np.stack([res.results[b]["out"].reshape(C, H, W) for b in range(B)])
    return out.astype(np.float32)


# revision 42
# speedup vs baseline: 1.1319x; 1.0047x over previous
"""Trainium2 Bass kernel for nn_CrossAttentionBlock (B=8, C=256, H=W=48).

Sharding: data-parallel over batch B — one batch per NeuronCore (8 cores).

Per-core math (x: [C=256, N=2304] f32):
  LayerNorm over C folded into projection weights on host:
      W_eff = W * w_n[None,:],  b_eff = b + W @ b_n
  attention SCALE folded into Wq_eff / bq_eff.
  v bias folded into the output bias (softmax rows sum to one, so
  attn@(v+bv) = attn@v + bv, hence bp_eff = bp + Wp@bv_eff).

  Activations are host-cast to bf16 and DMA'd as [32, N] partition strips
  (the DMA queues are descriptor-rate-bound; full-row descriptors maximize
  bytes per descriptor).  Stats: mean via a 1/C-ones bf16 matmul on x;
  mean-square via a Pool-computed x*x fed to a second ones-matmul.
  xn = (x-u)*rstd on the DVE, emitted as bf16.

  Attention is computed transposed:  St[m,n] = sum_o k[o,m] q[o,n]
  so softmax normalization runs over the *partition* axis m:
    - no row-max subtraction (logits bounded ~21, exp safe in f32)
    - P = exp(St) (ScalarE, PSUM->SBUF bf16 eviction)
    - rowsum[n] = sum_m P[m,n] via an M=1 ones-matmul accumulated across m
    - 1/rowsum applied AFTER the output projection (scaling commutes
      with Wp); the reciprocal row is partition-broadcast by the Pool
      engine and fused into the PSUM->SBUF eviction of the attention
      output, and the Wp projection + residual + DMA-out run per chunk
      inside the attention loop so there is no serial tail.
  v is produced directly transposed (vT[m,o] = sum_c xn2[c,m] WvT[c,o])
  so P.V contracts over m on partitions with zero PE transposes.
"""

import os
import sys
import types
import ctypes
import contextlib

sys.path.insert(0, "/opt/trn_rl_repo")

import numpy as np
import ml_dtypes

# ---------------------------------------------------------------------------
# NTFF profile hook stub (antenv.axon_hooks is absent in this container; the
# ctypes shim mirrors trn_agent_boot). Only used when tracing is requested.
# ---------------------------------------------------------------------------


def _ntff_profile_via_ctypes(so_path):
    try:
        lib = ctypes.CDLL(so_path)
    except OSError:
        return None
    if not hasattr(lib, "axon_start_nrt_profile"):
        return None
    lib.axon_start_nrt_profile.argtypes = [
        ctypes.POINTER(ctypes.c_int64),
        ctypes.c_size_t,
    ]
    lib.axon_start_nrt_profile.restype = ctypes.c_int64
    lib.axon_stop_nrt_profile.argtypes = [ctypes.c_char_p]
    lib.axon_stop_nrt_profile.restype = ctypes.c_int64

    @contextlib.contextmanager
    def _hook(output_dir, device_ids):
        import jax

        jax.devices()
        if device_ids:
            ids = (ctypes.c_int64 * len(device_ids))(*device_ids)
            rc = lib.axon_start_nrt_profile(ids, len(device_ids))
        else:
            rc = lib.axon_start_nrt_profile(None, 0)
        if rc != 0:
            raise RuntimeError(f"axon_start_nrt_profile rc={rc}")
        try:
            yield
        finally:
            n = lib.axon_stop_nrt_profile(str(output_dir).encode())
            print(f"profile: {n} file(s) written to {output_dir}", file=sys.stderr)

    return _hook


if "antenv.axon_hooks" not in sys.modules:
    _hook = _ntff_profile_via_ctypes("/opt/axon/libaxon_pjrt.so")
    _mod = types.ModuleType("antenv.axon_hooks")
    _mod.get_axon_ntff_profile_hook = lambda: _hook
    sys.modules["antenv.axon_hooks"] = _mod

# ---------------------------------------------------------------------------

B, C, H, W = 8, 256, 48, 48
N = H * W  # 2304
SCALE = (C // 8) ** (-0.5)
EPS = 1e-6
CT = C // 128  # 2 channel tiles
MT = N // 128  # 18 m (key-token) tiles
CHUNKS = [(0, 512), (512, 512), (1024, 512), (1536, 512), (2048, 256)]
NJ = len(CHUNKS)

BF16 = ml_dtypes.bfloat16

_cache = {}
last_results = None  # BassKernelResults of the most recent run (for test.py)


def _build_program():
    import concourse.bacc as bacc
    import concourse.tile as tile
    import concourse.mybir as mybir
    from contextlib import ExitStack

    f32 = mybir.dt.float32
    bf16 = mybir.dt.bfloat16
    ADD = mybir.AluOpType.add
    SUB = mybir.AluOpType.subtract

    nc = bacc.Bacc("TRN2", target_bir_lowering=False, debug=False)

    x1_d = nc.dram_tensor("x1", [C, N], bf16, kind="ExternalInput").ap()
    x2_d = nc.dram_tensor("x2", [C, N], bf16, kind="ExternalInput").ap()
    wqt_d = nc.dram_tensor("wqt", [C, C], bf16, kind="ExternalInput").ap()
    wkt_d = nc.dram_tensor("wkt", [C, C], bf16, kind="ExternalInput").ap()
    wvt_d = nc.dram_tensor("wvt", [C, C], bf16, kind="ExternalInput").ap()
    wpt_d = nc.dram_tensor("wpt", [C, C], bf16, kind="ExternalInput").ap()
    # cvec columns: 0/1 = bq per o-tile, 2/3 = bk per o-tile, 4/5 = bp_eff per
    # c-tile.  (The 1/C and ones constant blocks are memset on-device.)
    cvec_d = nc.dram_tensor("cvec", [128, 6], f32, kind="ExternalInput").ap()
    out_d = nc.dram_tensor("out", [C, N], f32, kind="ExternalOutput").ap()

    # m-tiles covered by each chunk: chunk j covers m in [off/128, (off+w)/128)
    def chunk_mtiles(ji):
        off, w = CHUNKS[ji]
        return range(off // 128, (off + w) // 128)

    with tile.TileContext(nc) as tc, ExitStack() as ctx:
        persist = ctx.enter_context(tc.tile_pool(name="persist", bufs=1))

        # DMA notes: (1) each descriptor covers one partition-row segment and
        # the queues are descriptor-rate-bound (~50ns each), so loads are
        # split into partition strips (full rows = max bytes per descriptor);
        # (2) each dma_start costs ~600ns of *issue* time on its engine's
        # sequencer, so issues are spread across the four idle sequencers.
        def strip_load(eng, dst, src, prows):
            P = dst.shape[0]
            for p in range(0, P, prows):
                pe = min(P, p + prows)
                eng.dma_start(dst[p:pe, :], src[p:pe, :])

        # ---- constants built on-device (no DMA) ------------------------
        invC = persist.tile([128, 128], bf16, tag="invC", name="invC")
        nc.vector.memset(invC[:], 1.0 / C)
        onesb = persist.tile([128, 128], bf16, tag="onesb", name="onesb")
        nc.gpsimd.memset(onesb[:], 1.0)

        # ---- x2 first: it gates k/v -> attention -----------------------
        x2sc = ctx.enter_context(tc.tile_pool(name="x2scope", bufs=1))
        x2_t = [
            x2sc.tile([128, N], bf16, tag=f"x2_{ct}", name=f"x2_{ct}")
            for ct in range(CT)
        ]
        strip_load(nc.sync, x2_t[0], x2_d[0:128, :], 64)
        strip_load(nc.scalar, x2_t[1], x2_d[128:256, :], 64)

        w_tiles = {}
        for (nm, d), eng in ((("k", wkt_d), nc.sync), (("v", wvt_d), nc.scalar)):
            for ct in range(CT):
                t = persist.tile([128, C], bf16, tag=f"w{nm}{ct}", name=f"w{nm}{ct}")
                strip_load(eng, t, d[ct * 128 : (ct + 1) * 128, :], 64)
                w_tiles[(nm, ct)] = t
        cvec = persist.tile([128, 6], f32, tag="cvec", name="cvec")
        nc.sync.dma_start(cvec[:], cvec_d[:, :])

        x1_t = [
            persist.tile([128, N], bf16, tag=f"x1_{ct}", name=f"x1_{ct}")
            for ct in range(CT)
        ]
        strip_load(nc.sync, x1_t[0], x1_d[0:128, :], 64)
        strip_load(nc.scalar, x1_t[1], x1_d[128:256, :], 64)

        for nm, d in (("q", wqt_d), ("p", wpt_d)):
            for ct in range(CT):
                t = persist.tile([128, C], bf16, tag=f"w{nm}{ct}", name=f"w{nm}{ct}")
                nc.gpsimd.dma_start(t[:], d[ct * 128 : (ct + 1) * 128, :])
                w_tiles[(nm, ct)] = t

        # persistent intermediates
        k_t = [
            persist.tile([128, N], bf16, tag=f"k{ot}", name=f"k{ot}")
            for ot in range(CT)
        ]
        vT_t = [
            persist.tile([128, C], bf16, tag=f"vT{m}", name=f"vT{m}")
            for m in range(MT)
        ]
        xn1_t = [
            persist.tile([128, N], bf16, tag=f"xn1_{ct}", name=f"xn1_{ct}")
            for ct in range(CT)
        ]

        # ------------------------------------------------------------------
        # Pre-phase: per-chunk pipeline  stats -> xn -> k/vT   (x2 stream)
        # plus the x1 stats/xn stream (feeds q projections later).
        # ------------------------------------------------------------------
        scr = ctx.enter_context(tc.tile_pool(name="scr", bufs=4))
        xnp = ctx.enter_context(tc.tile_pool(name="xnp", bufs=6))
        with (
            tc.tile_pool(name="ps_st", bufs=2, space="PSUM") as ps_st,
            tc.tile_pool(name="ps_kv", bufs=2, space="PSUM") as ps_kv,
        ):

            def emit_stats_xn(tsel, ji, xsrc, xn_out, stpool=None):
                """stats + xn for (tensor tsel, chunk ji).

                xsrc: list of [128, N] bf16 tiles (per ct)
                xn_out: dict key (ct) -> (tile, col_off) destination slices
                """
                off, w = CHUNKS[ji]
                pool = stpool if stpool is not None else ps_st
                tg = "pj" if stpool is not None else "ub"
                ub = pool.tile([128, 512], f32, tag=tg, name="ub")
                for ct in range(CT):
                    nc.tensor.matmul(
                        ub[:, :w],
                        invC[:],
                        xsrc[ct][:, off : off + w],
                        start=(ct == 0),
                        stop=(ct == CT - 1),
                    )
                tg = "pj" if stpool is not None else "ms"
                ms = pool.tile([128, 512], f32, tag=tg, name="ms")
                for ct in range(CT):
                    xsq = scr.tile([128, 512], bf16, tag="xsq", name="xsq")
                    nc.gpsimd.tensor_mul(
                        xsq[:, :w],
                        xsrc[ct][:, off : off + w],
                        xsrc[ct][:, off : off + w],
                    )
                    nc.tensor.matmul(
                        ms[:, :w],
                        invC[:],
                        xsq[:, :w],
                        start=(ct == 0),
                        stop=(ct == CT - 1),
                    )
                usq = scr.tile([128, 512], f32, tag="usq", name="usq")
                nc.scalar.square(usq[:, :w], ub[:, :w])
                var = scr.tile([128, 512], f32, tag="var", name="var")
                nc.vector.scalar_tensor_tensor(
                    var[:, :w], ms[:, :w], EPS, usq[:, :w], ADD, SUB
                )
                std = scr.tile([128, 512], f32, tag="std", name="std")
                nc.scalar.activation(
                    std[:, :w], var[:, :w], mybir.ActivationFunctionType.Sqrt
                )
                rstd = scr.tile([128, 512], f32, tag=f"rstd{tsel}", name=f"rstd{tsel}")
                nc.vector.reciprocal_approx_fast(rstd[:, :w], std[:, :w])
                for ct in range(CT):
                    d = scr.tile([128, 512], f32, tag="xnd", name="xnd")
                    nc.vector.tensor_sub(
                        d[:, :w], xsrc[ct][:, off : off + w], ub[:, :w]
                    )
                    dst, dcol = xn_out[ct]
                    # x1's multiplies go to the otherwise-idle Pool engine
                    eng = nc.vector if tsel == 1 else nc.gpsimd
                    eng.tensor_mul(dst[:, dcol : dcol + w], d[:, :w], rstd[:, :w])

            xn2 = {}

            def emit_kv(ji):
                off, w = CHUNKS[ji]
                # k projection for this chunk of tokens
                for ot in range(CT):
                    ps = ps_kv.tile([128, 512], f32, tag="kv", name="kv")
                    for ct in range(CT):
                        nc.tensor.matmul(
                            ps[:, :w],
                            w_tiles[("k", ct)][:, ot * 128 : (ot + 1) * 128],
                            xn2[(ji, ct)][:, :w],
                            start=(ct == 0),
                            stop=(ct == CT - 1),
                        )
                    nc.vector.tensor_scalar_add(
                        k_t[ot][:, off : off + w], ps[:, :w], cvec[:, 2 + ot : 3 + ot]
                    )
                # vT for the m-tiles inside this chunk
                for m in chunk_mtiles(ji):
                    coff = m * 128 - off
                    ps = ps_kv.tile([128, C], f32, tag="kv", name="kv")
                    for ct in range(CT):
                        nc.tensor.matmul(
                            ps[:],
                            xn2[(ji, ct)][:, coff : coff + 128],
                            w_tiles[("v", ct)][:, :],
                            start=(ct == 0),
                            stop=(ct == CT - 1),
                        )
                    nc.scalar.copy(vT_t[m][:], ps[:])

            # x2 stream first (it gates the attention m-loop chunk by chunk);
            # x1 chunks 0/1 next (chunk 0 gates qproj(0) and attention
            # start); x1 chunks 2-4 are emitted inside the attention loop,
            # two chunks ahead of their consumer, so they sit at the right
            # priority altitude and fill bubbles without starving anyone.
            def emit_x1(ji, stpool=None):
                emit_stats_xn(
                    0, ji, x1_t,
                    {ct: (xn1_t[ct], CHUNKS[ji][0]) for ct in range(CT)},
                    stpool=stpool,
                )

            for ji in range(NJ):
                for ct in range(CT):
                    t = xnp.tile([128, 512], bf16, tag="xn2", name=f"xn2_{ji}_{ct}")
                    xn2[(ji, ct)] = t
                emit_stats_xn(1, ji, x2_t, {ct: (xn2[(ji, ct)], 0) for ct in range(CT)})
                emit_kv(ji)
                if ji == 0:
                    emit_x1(0)
            emit_x1(1)

        # ------------------------------------------------------------------
        # Attention: per q-chunk; q projected one chunk ahead; epilogue
        # (normalize, Wp projection, residual, DMA out) inside the loop.
        # x1 chunks 2-4's stats are emitted inside the loop, two chunks
        # ahead of their consumer, borrowing the ps_pj pool for their PSUM.
        # ------------------------------------------------------------------
        with (
            tc.tile_pool(name="qch", bufs=4) as qch,
            tc.tile_pool(name="pt", bufs=24) as pt_pool,
            tc.tile_pool(name="oup", bufs=4) as oup,
            tc.tile_pool(name="invp", bufs=2) as invp,
            tc.tile_pool(name="outp", bufs=4) as outp,
            tc.tile_pool(name="ps_pj", bufs=2, space="PSUM") as ps_pj,
            tc.tile_pool(name="ps_qk", bufs=2, space="PSUM") as ps_qk,
            tc.tile_pool(name="ps_o", bufs=3, space="PSUM") as ps_o,
            tc.tile_pool(name="ps_rs", bufs=1, space="PSUM") as ps_rs,
        ):
            q_ch = {}

            def emit_qproj(ji):
                off, w = CHUNKS[ji]
                for ot in range(CT):
                    ps = ps_pj.tile([128, 512], f32, tag="pj", name="pj")
                    for ct in range(CT):
                        nc.tensor.matmul(
                            ps[:, :w],
                            w_tiles[("q", ct)][:, ot * 128 : (ot + 1) * 128],
                            xn1_t[ct][:, off : off + w],
                            start=(ct == 0),
                            stop=(ct == CT - 1),
                        )
                    qt = qch.tile([128, 512], bf16, tag="q", name=f"q{ji}_{ot}")
                    nc.vector.tensor_scalar_add(
                        qt[:, :w], ps[:, :w], cvec[:, 0 + ot : 1 + ot]
                    )
                    q_ch[(ji, ot)] = qt

            emit_qproj(0)
            for ji, (off, w) in enumerate(CHUNKS):
                if ji + 1 < NJ:
                    emit_qproj(ji + 1)
                if ji + 2 < NJ:
                    emit_x1(ji + 2, stpool=ps_pj)
                st = {}

                def emit_qk(m):
                    ps = ps_qk.tile([128, 512], f32, tag="st", name="st")
                    for ot in range(CT):
                        nc.tensor.matmul(
                            ps[:, :w],
                            k_t[ot][:, m * 128 : (m + 1) * 128],
                            q_ch[(ji, ot)][:, :w],
                            start=(ot == 0),
                            stop=(ot == CT - 1),
                        )
                    st[m] = ps

                o_ps = [
                    ps_o.tile([128, 512], f32, tag="o", name="o") for _ in range(CT)
                ]

                # Rowsum: last chunk interleaves per-m ones-matmuls (keeps
                # the tail short); other chunks reduce the pt tiles with a
                # 2-level bf16 pair/quad tree on the otherwise-idle Pool
                # engine, leaving only 5 ones-matmuls per chunk on the PE.
                inline_rs = ji == NJ - 1
                rs_ps = ps_rs.tile([128, 512], f32, tag="rsp", name="rsp")

                pts = []

                emit_qk(0)
                for m in range(MT):
                    if m + 1 < MT:
                        emit_qk(m + 1)
                    pt = pt_pool.tile([128, 512], bf16, tag="pt", name=f"pt{m}")
                    nc.scalar.activation(
                        pt[:, :w], st[m][:, :w], mybir.ActivationFunctionType.Exp
                    )
                    del st[m]
                    pts.append(pt)
                    for c in range(CT):
                        nc.tensor.matmul(
                            o_ps[c][:, :w],
                            vT_t[m][:, c * 128 : (c + 1) * 128],
                            pt[:, :w],
                            start=(m == 0),
                            stop=(m == MT - 1),
                        )
                    if inline_rs:
                        nc.tensor.matmul(
                            rs_ps[:, :w],
                            onesb[:, 0:128],
                            pt[:, :w],
                            start=(m == 0),
                            stop=(m == MT - 1),
                        )

                # ---- chunk epilogue -----------------------------------
                if not inline_rs:
                    for i, t in enumerate(pts):
                        nc.tensor.matmul(
                            rs_ps[:, :w],
                            onesb[:, 0:128],
                            t[:, :w],
                            start=(i == 0),
                            stop=(i == len(pts) - 1),
                        )
                inv_b = invp.tile([128, 512], f32, tag="invb", name="invb")
                nc.vector.reciprocal_approx_fast(inv_b[:, :w], rs_ps[:, :w])

                ou = []
                for c in range(CT):
                    t = oup.tile([128, 512], bf16, tag="ou", name=f"ou{c}")
                    nc.vector.tensor_mul(t[:, :w], o_ps[c][:, :w], inv_b[:, :w])
                    ou.append(t)

                for ct in range(CT):
                    ps = ps_pj.tile([128, 512], f32, tag="pj", name="pj")
                    for ci in range(CT):
                        nc.tensor.matmul(
                            ps[:, :w],
                            w_tiles[("p", ci)][:, ct * 128 : (ct + 1) * 128],
                            ou[ci][:, :w],
                            start=(ci == 0),
                            stop=(ci == CT - 1),
                        )
                    ot_t = outp.tile([128, 512], f32, tag="outt", name=f"out{ct}")
                    nc.vector.scalar_tensor_tensor(
                        ot_t[:, :w],
                        ps[:, :w],
                        cvec[:, 4 + ct : 5 + ct],
                        x1_t[ct][:, off : off + w],
                        ADD,
                        ADD,
                    )
                    if ji + 1 < NJ:
                        nc.sync.dma_start(
                            out_d[ct * 128 : (ct + 1) * 128, off : off + w],
                            ot_t[:, :w],
                        )
                    else:
                        # last chunk is latency-critical: strip across queues
                        # and split the issue cost across two sequencers
                        eng = nc.sync if ct == 0 else nc.scalar
                        for p in range(0, 128, 64):
                            eng.dma_start(
                                out_d[ct * 128 + p : ct * 128 + p + 64, off : off + w],
                                ot_t[p : p + 64, :w],
                            )

    nc.compile()
    return nc


def _host_prep(inputs):
    f = lambda k: np.asarray(inputs[k], dtype=np.float32)
    Wq, Wk, Wv, Wp = f("Wq"), f("Wk"), f("Wv"), f("Wp")
    bq, bk, bv, bp = f("bq"), f("bk"), f("bv"), f("bp")
    w_nq, b_nq, w_nkv, b_nkv = f("w_nq"), f("b_nq"), f("w_nkv"), f("b_nkv")

    Wq_eff = Wq * w_nq[None, :] * SCALE
    bq_eff = SCALE * (bq + Wq @ b_nq)
    Wk_eff = Wk * w_nkv[None, :]
    bk_eff = bk + Wk @ b_nkv
    Wv_eff = Wv * w_nkv[None, :]
    bv_eff = bv + Wv @ b_nkv
    bp_eff = bp + Wp @ bv_eff  # v bias folded through softmax + Wp

    wqt = np.ascontiguousarray(Wq_eff.T).astype(BF16)
    wkt = np.ascontiguousarray(Wk_eff.T).astype(BF16)
    wvt = np.ascontiguousarray(Wv_eff.T).astype(BF16)
    wpt = np.ascontiguousarray(Wp.T).astype(BF16)

    cvec = np.zeros((128, 6), np.float32)
    cvec[:, 0] = bq_eff[0:128]
    cvec[:, 1] = bq_eff[128:256]
    cvec[:, 2] = bk_eff[0:128]
    cvec[:, 3] = bk_eff[128:256]
    cvec[:, 4] = bp_eff[0:128]
    cvec[:, 5] = bp_eff[128:256]

    return dict(wqt=wqt, wkt=wkt, wvt=wvt, wpt=wpt, cvec=cvec)


def _maybe_patch_ldw_opt():
    if os.environ.get("BASS_LDW_OPT", "0") != "1":
        return
    import concourse.bass_utils as bu
    if getattr(bu, "_ldw_patch", False):
        return
    orig = bu.run_command
    def patched(argv, **kw):
        if isinstance(argv, list):
            argv = [a.replace("--enable-ldw-opt=false", "--enable-ldw-opt=true") for a in argv]
        return orig(argv, **kw)
    bu.run_command = patched
    bu._ldw_patch = True


def kernel(**inputs):
    global last_results
    _maybe_patch_ldw_opt()
    from concourse.bass_utils import run_bass_kernel_spmd

    if "nc" not in _cache:
        _cache["nc"] = _build_program()
    nc = _cache["nc"]

    shared = _host_prep(inputs)
    x1 = np.asarray(inputs["x1"], dtype=np.float32).reshape(B, C, N).astype(BF16)
    x2 = np.asarray(inputs["x2"], dtype=np.float32).reshape(B, C, N).astype(BF16)

    in_maps = []
    for b in range(B):
        m = dict(shared)
        m["x1"] = np.ascontiguousarray(x1[b])
        m["x2"] = np.ascontiguousarray(x2[b])
        in_maps.append(m)

    trace = os.environ.get("BASS_KERNEL_TRACE", "0") == "1"
    res = run_bass_kernel_spmd(
        nc, in_maps, core_ids=list(range(B)), trace=trace
    )
    last_results = res
    out = np.stack([res.results[b]["out"].reshape(C, H, W) for b in range(B)])
    return out.astype(np.float32)
